# revision 2
# baseline (speedup 1.0000x reference)
"""Trainium2 Bass kernel for DeepGCN (nn_DeepGCN_82454782148693), v2.

Key changes vs v1 baseline (10.5 ms):
  - Host-precomputed S matrices (bf16, norm folded in) streamed from DRAM
    replace on-device DVE is_equal one-hot generation (was 9.5 ms DVE).
  - dma_gather calls round-robin over 4 SWDGE queues (desc-gen
    parallelizes: 8.9 -> 3.5 ns/idx on the Q7).
  - Aggregation runs feature-major: matmul(yT += msgs^T @ S) with the
    gathered messages as the stationary operand; no PE transposes.
  - hall / messages / S in bf16 (PSUM accumulate f32): halves gather and
    AllGather bytes.
  - Self-loops applied as identity matmuls from an SBUF-resident
    dinv^2-scaled copy of h' (saves 12.5k gather idxs/layer).
  - AllGather split into 4 node-quarters, each fired as soon as P1
    produces its rows (overlaps collective with compute); quarters are
    also the int16 gather banks.
  - h2 kept resident in SBUF (bf16); no DRAM round-trip per layer.

Schedule is static and shared across cores: per (tile, bank) buckets are
padded to K(t,b) = max over cores of ceil(count/128) chunks.
"""

import os
import sys

import numpy as np

for _p in ("/opt/trn_rl_repo", "/root/.axon_site/_ro/trn_rl_repo"):
    if os.path.isdir(_p) and _p not in sys.path:
        sys.path.append(_p)

import ml_dtypes
import concourse.bass as bass
import concourse.bacc as bacc
import concourse.mybir as mybir
import concourse.tile as tile
from concourse import bass_utils

F32 = mybir.dt.float32
BF16 = mybir.dt.bfloat16
I16 = mybir.dt.int16
AF = mybir.ActivationFunctionType
OP = mybir.AluOpType
AX = mybir.AxisListType


class Cfg:
    def __init__(self):
        self.N, self.E, self.NCORES = 100000, 1600000, 8
        self.H, self.L, self.HC, self.C = 128, 4, 64, 2
        self.ALPHA, self.THETA, self.EPS = 0.1, 0.5, 1e-5
        self.NSH = self.N // self.NCORES            # 12500
        self.TILES = (self.NSH + 127) // 128        # 98
        self.NPAD = self.TILES * 128                # 12544
        # quarters (AllGather chunks == gather banks), tile-aligned
        self.QTILES = [25, 25, 24, 24]
        self.QSTART = [0, 3200, 6400, 9472]
        self.QSIZE = [3200, 3200, 3072, 3072]
        self.NB = 4                                  # banks
        self.WPG = 4                                 # windows per group
        self.NW = (self.TILES + 3) // 4              # 25 windows
        self.NG = (self.NW + self.WPG - 1) // self.WPG   # 5 groups
        self.UNIT = 8                                # chunks per gather call
        self.NQ = int(os.environ.get("GCN_NQ", "4"))  # swdge queues


CFG = Cfg()


# ----------------------------------------------------------------------------
# Host preprocessing
# ----------------------------------------------------------------------------

def build_host_data(edge_index, cfg, perms=None):
    c = cfg
    src = edge_index[0].astype(np.int64)
    dst = edge_index[1].astype(np.int64)

    deg = (np.bincount(dst, minlength=c.N) + 1.0).astype(np.float64)
    dinv = (1.0 / np.sqrt(deg)).astype(np.float32)
    norm = (dinv[src] * dinv[dst]).astype(np.float32)

    core = dst // c.NSH
    dloc = dst - core * c.NSH
    s_core = src // c.NSH
    s_lp = src - s_core * c.NSH
    if perms is not None:
        invs = np.empty((c.NCORES, c.NSH), np.int64)
        for k in range(c.NCORES):
            invs[k, perms[k]] = np.arange(c.NSH)
        s_lp = invs[s_core, s_lp]
        dloc = invs[core, dloc]
    t = dloc >> 7
    doff = dloc & 127
    q = np.digitize(s_lp, c.QSTART[1:])              # bank 0..3
    qsz = np.array(c.QSIZE, np.int64)
    qst = np.array(c.QSTART, np.int64)
    brow = s_core * qsz[q] + s_lp - qst[q]

    # K(t,b): shared chunk counts
    cnt = np.zeros((c.NCORES, c.TILES, c.NB), np.int64)
    np.add.at(cnt, (core, t, q), 1)
    K = np.maximum(1, -(-cnt.max(axis=0) // 128)).astype(np.int64)  # [98,4]

    # emission order: group -> bank -> tile -> chunk
    tiles_of_group = []
    for g in range(c.NG):
        w0, w1 = g * c.WPG, min((g + 1) * c.WPG, c.NW)
        tiles_of_group.append(list(range(w0 * 4, min(w1 * 4, c.TILES))))
    base = np.zeros((c.TILES, c.NB), np.int64)
    ctr = 0
    for g in range(c.NG):
        for b in range(c.NB):
            for tt in tiles_of_group[g]:
                base[tt, b] = ctr
                ctr += K[tt, b]
    NCH = ctr

    # slot position of each edge within its (core,t,b) bucket
    okey = (core * c.TILES + t) * c.NB + q
    order = np.argsort(okey, kind="stable")
    ko = okey[order]
    runstart = np.r_[0, np.flatnonzero(np.diff(ko)) + 1]
    runid = np.zeros(len(ko), np.int64)
    runid[runstart[1:]] = 1
    runid = np.cumsum(runid)
    pos = np.arange(len(ko)) - runstart[runid]
    inv = np.empty_like(order)
    inv[order] = np.arange(len(order))
    pos = pos[inv]                                   # per original edge

    gchunk = base[t, q] + (pos >> 7)                 # global chunk id
    p = pos & 127                                    # slot in chunk

    idx_slots = np.zeros((c.NCORES, NCH * 128), np.int16)
    S_host = np.zeros((c.NCORES, 128, NCH * 128), ml_dtypes.bfloat16)
    idx_slots[core, gchunk * 128 + p] = brow.astype(np.int16)
    S_host[core, p, gchunk * 128 + doff] = norm

    # wrap idx in 16 partitions, replicate to 128
    idx_w = idx_slots.reshape(c.NCORES, NCH * 8, 16).transpose(0, 2, 1)
    idx_in = np.tile(idx_w, (1, 8, 1)).astype(np.int16)  # [NCORES,128,NCH*8]

    # dinv^2 per local node, [NCORES, 128, TILES] (0 on pad rows)
    d2 = np.zeros((c.NCORES, c.NPAD), np.float32)
    d2r = (1.0 / deg).astype(np.float32).reshape(c.NCORES, c.NSH)
    if perms is not None:
        d2r = np.stack([d2r[k][perms[k]] for k in range(c.NCORES)])
    d2[:, :c.NSH] = d2r
    dinv2_in = d2.reshape(c.NCORES, c.TILES, 128).transpose(0, 2, 1).copy()

    return K, base, NCH, idx_in, S_host, dinv2_in, tiles_of_group


def pack_weights(inputs, cfg):
    c = cfg
    H, L, HC, Cc = c.H, c.L, c.HC, c.C
    cols = [np.asarray(inputs["proj_W"], np.float32)]
    for l in range(L):
        cols.append(np.asarray(inputs["conv_W"][l], np.float32))
    for l in range(L):
        cols.append(np.asarray(inputs["lin_W"][l], np.float32))
    cols.append(np.asarray(inputs["cls_W1"], np.float32))
    w2 = np.zeros((H, Cc), np.float32)
    w2[:HC] = np.asarray(inputs["cls_W2"], np.float32)
    cols.append(w2)
    W = np.concatenate(cols, axis=1)

    nb = np.zeros((H, 19), np.float32)
    nb[:, 0] = np.asarray(inputs["proj_b"], np.float32)
    for l in range(L):
        nb[:, 1 + l] = np.asarray(inputs["conv_b"][l], np.float32)
        nb[:, 5 + l] = np.asarray(inputs["lin_b"][l], np.float32)
        nb[:, 9 + l] = np.asarray(inputs["bn_g"][l], np.float32)
        nb[:, 13 + l] = np.asarray(inputs["bn_b"][l], np.float32)
    nb[:HC, 17] = np.asarray(inputs["cls_b1"], np.float32)
    nb[:Cc, 18] = np.asarray(inputs["cls_b2"], np.float32)
    return W, nb


# ----------------------------------------------------------------------------
# Device program
# ----------------------------------------------------------------------------

def build_program(cfg, K, base, NCH, tiles_of_group, debug=False):
    c = cfg
    H, L = c.H, c.L
    WCOLS = 128 * (1 + 2 * L) + c.HC + c.C
    C1 = float(1.0 - c.ALPHA - c.THETA)

    nc = bacc.Bacc(
        "TRN2",
        target_bir_lowering=False,
        debug=False,
        enable_asserts=False,
        num_devices=c.NCORES,
        num_swdge_queues=c.NQ,
    )

    # ---- I/O ----
    xT_in = nc.dram_tensor("xT_in", [H, c.NPAD], F32, kind="ExternalInput").ap()
    w_in = nc.dram_tensor("w_in", [H, WCOLS], F32, kind="ExternalInput").ap()
    b_in = nc.dram_tensor("b_in", [H, 19], F32, kind="ExternalInput").ap()
    idx_in = nc.dram_tensor("idx_in", [H, NCH * 8], I16,
                            kind="ExternalInput").ap()
    s_in = nc.dram_tensor("s_in", [H, NCH * 128], BF16,
                          kind="ExternalInput").ap()
    d2_in = nc.dram_tensor("d2_in", [H, c.TILES], F32,
                           kind="ExternalInput").ap()
    out_d = nc.dram_tensor("out_d", [c.C, c.NPAD], F32,
                           kind="ExternalOutput").ap()
    if debug:
        dbg_x = nc.dram_tensor("dbg_x", [H, c.NPAD], F32,
                               kind="ExternalOutput").ap()
        dbg_hnm = nc.dram_tensor("dbg_hnm", [H, c.NPAD], BF16,
                                 kind="ExternalOutput").ap()
        dbg_hall = nc.dram_tensor("dbg_hall", [c.NCORES * c.QSIZE[0], H],
                                  BF16, kind="ExternalOutput").ap()
        dbg_h2 = nc.dram_tensor("dbg_h2", [H, c.NPAD], BF16,
                                kind="ExternalOutput").ap()
        dbg_stat = nc.dram_tensor("dbg_stat", [H, 12], F32,
                                  kind="ExternalOutput").ap()
        dbg_x1 = nc.dram_tensor("dbg_x1", [H, c.NPAD], F32,
                                kind="ExternalOutput").ap()
        dbg_y = nc.dram_tensor("dbg_y", [H, c.NPAD], F32,
                               kind="ExternalOutput").ap()

    # ---- internal DRAM ----
    hsh = [nc.dram_tensor(f"hsh{qi}", [c.QSIZE[qi], H], BF16,
                          kind="Internal").ap() for qi in range(4)]
    hall = [nc.dram_tensor(f"hall{qi}", [c.NCORES * c.QSIZE[qi], H], BF16,
                           kind="Internal", addr_space="Shared").ap()
            for qi in range(4)]
    stin_d = nc.dram_tensor("stin_d", [H, 2], F32, kind="Internal").ap()
    stout_d = nc.dram_tensor("stout_d", [H, 2], F32, kind="Internal",
                             addr_space="Shared").ap()

    # ---- SBUF residents ----
    xT = nc.alloc_sbuf_tensor("xT", [H, c.NPAD], F32).ap()
    x0s = nc.alloc_sbuf_tensor("x0s", [H, c.NPAD], BF16).ap()
    h2T = nc.alloc_sbuf_tensor("h2T", [H, c.NPAD], BF16).ap()
    hnm = nc.alloc_sbuf_tensor("hnm", [H, c.NPAD], BF16).ap()  # dinv2-scaled
    wsb = nc.alloc_sbuf_tensor("wsb", [H, WCOLS], F32).ap()
    bsb = nc.alloc_sbuf_tensor("bsb", [H, 19], F32).ap()
    d2sb = nc.alloc_sbuf_tensor("d2sb", [H, c.TILES], F32).ap()
    identb = nc.alloc_sbuf_tensor("identb", [H, H], BF16).ap()
    sums = nc.alloc_sbuf_tensor("sums", [H, 32], F32).ap()
    sqs = nc.alloc_sbuf_tensor("sqs", [H, 32], F32).ap()
    stat = nc.alloc_sbuf_tensor("stat", [H, 12], F32).ap()

    wproj = wsb[:, 0:128]
    wconv = lambda l: wsb[:, 128 * (1 + l):128 * (2 + l)]
    wlin = lambda l: wsb[:, 128 * (1 + L + l):128 * (2 + L + l)]
    wcls1 = wsb[:, 128 * (1 + 2 * L):128 * (1 + 2 * L) + c.HC]
    wcls2 = wsb[:c.HC, 128 * (1 + 2 * L) + c.HC:WCOLS]

    rg = [list(range(c.NCORES))]

    # chunk -> (tile, is_last) in emission order, per group/bank
    stop_chunk = {}          # global chunk id -> True for (b=3, k=K-1)
    tile_of_chunk = {}
    for g in range(c.NG):
        for b in range(c.NB):
            for tt in tiles_of_group[g]:
                for k in range(K[tt, b]):
                    cid = base[tt, b] + k
                    tile_of_chunk[cid] = tt
                    stop_chunk[cid] = (b == c.NB - 1) and (k == K[tt, b] - 1)

    # node chunks of 512 for dense sweeps
    PCH = [(o, min(512, c.NPAD - o)) for o in range(0, c.NPAD, 512)]

    with tile.TileContext(nc) as tc:
        with tc.sbuf_pool(name="pio", bufs=3) as pio, \
             tc.sbuf_pool(name="pstg", bufs=3) as pstg, \
             tc.sbuf_pool(name="pidx", bufs=3) as pidx, \
             tc.sbuf_pool(name="pmsg", bufs=8) as pmsg, \
             tc.sbuf_pool(name="psb", bufs=4) as psb, \
             tc.sbuf_pool(name="pyt", bufs=3) as pyt, \
             tc.sbuf_pool(name="pscr", bufs=3) as pscr, \
             tc.sbuf_pool(name="pfin", bufs=4) as pfin, \
             tc.psum_pool(name="ppmm", bufs=2) as ppmm, \
             tc.psum_pool(name="ppagg", bufs=4) as ppagg, \
             tc.psum_pool(name="pplin", bufs=2) as pplin:

            # ================= P0: prologue =================
            nc.sync.dma_start(wsb, w_in)
            nc.sync.dma_start(bsb, b_in)
            nc.sync.dma_start(d2sb, d2_in)
            # identity (bf16)
            iota = pio.tile([H, H], F32, tag="iota")
            pidx_t = pio.tile([H, H], F32, tag="pidxt")
            nc.gpsimd.iota(iota, pattern=[[1, H]], base=0, channel_multiplier=0,
                           allow_small_or_imprecise_dtypes=True)
            nc.gpsimd.iota(pidx_t, pattern=[[0, H]], base=0,
                           channel_multiplier=1,
                           allow_small_or_imprecise_dtypes=True)
            nc.vector.tensor_tensor(identb, iota, pidx_t, OP.is_equal)
            # proj + relu -> xT f32 ; x0s = alpha*xT (bf16)
            for (off, w) in PCH:
                xin = pio.tile([H, 512], F32, tag="xin")
                nc.sync.dma_start(xin[:, :w], xT_in[:, off:off + w])
                ps = ppmm.tile([H, 512], F32, tag="ps")
                nc.tensor.matmul(ps[:, :w], wproj, xin[:, :w])
                nc.scalar.activation(xT[:, off:off + w], ps[:, :w], AF.Relu,
                                     bias=bsb[:, 0:1], scale=1.0)
                nc.vector.tensor_scalar_mul(x0s[:, off:off + w],
                                            xT[:, off:off + w], c.ALPHA)
            nc.vector.memset(xT[:, c.NSH:c.NPAD], 0.0)
            nc.vector.memset(x0s[:, c.NSH:c.NPAD], 0.0)
            nc.vector.memset(h2T[:, c.NSH:c.NPAD], 0.0)
            if debug:
                nc.sync.dma_start(dbg_x, xT)

            unit_ctr = 0

            # ================= layers =================
            for li in range(L):
                # ---- P1: h' = x @ convW, node-major -> hsh_q; hnm scaled ----
                qfire = {7: 0, 13: 1, 19: 2, 25: 3}
                for w4 in range(25):
                    t0 = w4 * 4
                    nt = min(4, c.TILES - t0)
                    ps = ppmm.tile([H, 512], F32, tag="ps")
                    for j in range(nt):
                        tt = t0 + j
                        nc.tensor.matmul(
                            ps[:, j * 128:(j + 1) * 128],
                            xT[:, tt * 128:(tt + 1) * 128], wconv(li))
                    stg = pstg.tile([H, 512], BF16, tag="stg")
                    nc.scalar.activation(stg[:, :nt * 128], ps[:, :nt * 128],
                                         AF.Identity)
                    for j in range(nt):
                        tt = t0 + j
                        nc.vector.tensor_scalar_mul(
                            hnm[:, tt * 128:(tt + 1) * 128],
                            ps[:, j * 128:(j + 1) * 128],
                            d2sb[:, tt:tt + 1])
                    # write node-major rows to the quarter tensors
                    r0 = t0 * 128
                    j0 = 0
                    while j0 < nt:
                        qi = np.digitize(r0 + j0 * 128, c.QSTART[1:])
                        qend = c.QSTART[qi] + c.QSIZE[qi]
                        jn = min(nt, (qend - r0) // 128)
                        rows = hsh[qi][r0 + j0 * 128 - c.QSTART[qi]:
                                       r0 + jn * 128 - c.QSTART[qi], :]
                        nc.sync.dma_start(
                            rows.rearrange("(j p) f -> p j f", p=128),
                            stg[:, j0 * 128:jn * 128].rearrange(
                                "p (j f) -> p j f", f=H))
                        j0 = jn
                    if w4 + 1 in qfire:
                        qi = qfire[w4 + 1]
                        nc.gpsimd.collective_compute(
                            "AllGather", OP.bypass, replica_groups=rg,
                            ins=[hsh[qi]], outs=[hall[qi]])
                if debug and li == 0:
                    nc.sync.dma_start(dbg_hnm, hnm)
                    nc.sync.dma_start(dbg_hall, hall[0])

                # ---- P3: gather + S matmuls + finalize ----
                for g in range(c.NG):
                    tg = tiles_of_group[g]
                    w0 = g * c.WPG
                    nw = min(c.WPG, c.NW - w0)
                    aggs = [ppagg.tile([H, 512], F32, tag="agg",
                                       name=f"agg_{li}_{g}_{wi}")
                            for wi in range(nw)]

                    # self-loops: first touch of every tile region.
                    # start=True resets the whole PSUM bank's written-bits,
                    # so only the window's FIRST matmul may carry it.
                    for tt in tg:
                        wi = tt // 4 - w0
                        toff = (tt % 4) * 128
                        nc.tensor.matmul(
                            aggs[wi][:, toff:toff + 128],
                            hnm[:, tt * 128:(tt + 1) * 128], identb,
                            start=(toff == 0), stop=False,
                            skip_group_check=True)

                    for b in range(c.NB):
                        c0 = base[tg[0], b]
                        cend = base[tg[-1], b] + K[tg[-1], b]
                        ngb = cend - c0
                        idxt = pidx.tile([H, 128 * 8], I16, tag="idxt")
                        nc.scalar.dma_start(
                            idxt[:, :ngb * 8],
                            idx_in[:, c0 * 8:c0 * 8 + ngb * 8])
                        cid = c0
                        while cid < cend:
                            nch = min(c.UNIT, cend - cid)
                            msgs = pmsg.tile([H, c.UNIT, H], BF16, tag="msgs")
                            nc.gpsimd.dma_gather(
                                out_ap=msgs[:, :nch, :],
                                in_ap=hall[b],
                                idxs_ap=idxt[:, (cid - c0) * 8:
                                             (cid - c0 + nch) * 8],
                                num_idxs=nch * 128,
                                num_idxs_reg=nch * 128,
                                elem_size=H,
                                queue_num=unit_ctr % c.NQ,
                            )
                            unit_ctr += 1
                            ssb = psb.tile([H, c.UNIT * 128], BF16, tag="ssb")
                            nc.sync.dma_start(
                                ssb[:, :nch * 128],
                                s_in[:, cid * 128:(cid + nch) * 128])
                            for ci in range(nch):
                                cc = cid + ci
                                tt = tile_of_chunk[cc]
                                wi = tt // 4 - w0
                                toff = (tt % 4) * 128
                                nc.tensor.matmul(
                                    aggs[wi][:, toff:toff + 128],
                                    msgs[:, ci, :],
                                    ssb[:, ci * 128:(ci + 1) * 128],
                                    start=False, stop=stop_chunk[cc],
                                    skip_group_check=True)
                            cid += nch

                    # finalize windows of this group
                    for wi in range(nw):
                        w = w0 + wi
                        col0 = w * 512
                        wcols = min(512, c.NPAD - col0)
                        yT = pyt.tile([H, 512], F32, tag="yT")
                        nc.scalar.activation(yT[:, :wcols], aggs[wi][:, :wcols],
                                             AF.Identity,
                                             bias=bsb[:, 1 + li:2 + li])
                        if debug and li == 0:
                            nc.sync.dma_start(dbg_y[:, col0:col0 + wcols],
                                              yT[:, :wcols])
                        ps2 = pplin.tile([H, 512], F32, tag="ps2")
                        nc.tensor.matmul(ps2[:, :wcols], wlin(li),
                                         yT[:, :wcols])
                        ecols = min(wcols, max(0, c.NSH - col0))
                        nc.vector.tensor_scalar(
                            h2T[:, col0:col0 + ecols], ps2[:, :ecols],
                            bsb[:, 5 + li:6 + li], None, op0=OP.add,
                            op1=OP.add, accum_out=sums[:, w:w + 1])
                        scr = pscr.tile([H, 512], BF16, tag="scr")
                        nc.vector.scalar_tensor_tensor(
                            scr[:, :ecols], h2T[:, col0:col0 + ecols], 0.0,
                            h2T[:, col0:col0 + ecols],
                            op0=OP.add, op1=OP.mult,
                            accum_out=sqs[:, w:w + 1])

                if debug and li == 0:
                    nc.sync.dma_start(dbg_h2, h2T)
                # ---- P4: BN stats ----
                nc.vector.tensor_reduce(stat[:, 0:1], sums[:, :c.NW], AX.X,
                                        OP.add)
                nc.vector.tensor_reduce(stat[:, 1:2], sqs[:, :c.NW], AX.X,
                                        OP.add)
                nc.sync.dma_start(stin_d, stat[:, 0:2])
                nc.gpsimd.collective_compute(
                    "AllReduce", OP.add, replica_groups=rg,
                    ins=[stin_d], outs=[stout_d])
                nc.sync.dma_start(stat[:, 4:6], stout_d)   # [S1, S2]
                invn = 1.0 / float(c.N)
                nc.vector.tensor_scalar_mul(stat[:, 6:7], stat[:, 4:5], invn)
                m2 = pfin.tile([H, 1], F32, tag="m2")
                nc.vector.tensor_tensor(m2, stat[:, 6:7], stat[:, 6:7],
                                        OP.mult)
                nc.vector.scalar_tensor_tensor(stat[:, 7:8], stat[:, 5:6],
                                               invn, m2, op0=OP.mult,
                                               op1=OP.subtract)
                vps = pfin.tile([H, 1], F32, tag="vps")
                nc.vector.tensor_scalar_add(vps, stat[:, 7:8], float(c.EPS))
                sd = pfin.tile([H, 1], F32, tag="sd")
                nc.scalar.sqrt(sd, vps)
                inv = pfin.tile([H, 1], F32, tag="inv")
                nc.vector.reciprocal(inv, sd)
                gi = pfin.tile([H, 1], F32, tag="gi")
                nc.vector.tensor_tensor(gi, inv, bsb[:, 9 + li:10 + li],
                                        OP.mult)
                nc.vector.tensor_scalar_mul(stat[:, 8:9], gi, C1)
                ms = pfin.tile([H, 1], F32, tag="ms")
                nc.vector.tensor_tensor(ms, stat[:, 6:7], stat[:, 8:9],
                                        OP.mult)
                nc.vector.scalar_tensor_tensor(stat[:, 9:10],
                                               bsb[:, 13 + li:14 + li], C1,
                                               ms, op0=OP.mult,
                                               op1=OP.subtract)

                # ---- P5: x = relu(s*h2 + u + alpha*x0 + theta*x_prev) ----
                for (off, w) in PCH:
                    t1 = pscr.tile([H, 512], F32, tag="t1", bufs=2)
                    nc.vector.tensor_scalar(t1[:, :w], h2T[:, off:off + w],
                                            stat[:, 8:9], stat[:, 9:10],
                                            op0=OP.mult, op1=OP.add)
                    t2 = pscr.tile([H, 512], F32, tag="t2", bufs=2)
                    nc.vector.scalar_tensor_tensor(t2[:, :w],
                                                   xT[:, off:off + w],
                                                   float(c.THETA), t1[:, :w],
                                                   op0=OP.mult, op1=OP.add)
                    t3 = pscr.tile([H, 512], F32, tag="t3", bufs=2)
                    nc.vector.tensor_tensor(t3[:, :w], t2[:, :w],
                                            x0s[:, off:off + w], OP.add)
                    nc.vector.tensor_scalar_max(xT[:, off:off + w], t3[:, :w],
                                                0.0)
                nc.vector.memset(xT[:, c.NSH:c.NPAD], 0.0)
                if debug and li == 0:
                    nc.sync.dma_start(dbg_stat, stat)
                    nc.sync.dma_start(dbg_x1, xT)

            # ================= P6: classifier =================
            for (off, w) in PCH:
                ps = ppmm.tile([H, 512], F32, tag="ps")
                nc.tensor.matmul(ps[:c.HC, :w], wcls1, xT[:, off:off + w])
                h3 = pio.tile([c.HC, 512], F32, tag="h3")
                nc.scalar.activation(h3[:, :w], ps[:c.HC, :w], AF.Relu,
                                     bias=bsb[:c.HC, 17:18], scale=1.0)
                ps2 = pplin.tile([H, 512], F32, tag="ps2")
                nc.tensor.matmul(ps2[:c.C, :w], wcls2, h3[:, :w])
                ot = pio.tile([c.C, 512], F32, tag="ot")
                nc.vector.tensor_scalar(ot[:, :w], ps2[:c.C, :w],
                                        bsb[:c.C, 18:19], None, op0=OP.add)
                nc.sync.dma_start(out_d[:, off:off + w], ot[:, :w])

    nc.compile()
    return nc


# ----------------------------------------------------------------------------
# Full pipeline
# ----------------------------------------------------------------------------

LAST_RESULTS = None
_PROGRAM_CACHE = {}


def kernel(**inputs):
    global LAST_RESULTS
    c = CFG
    x = np.ascontiguousarray(np.asarray(inputs["x"], np.float32))
    edge_index = np.asarray(inputs["edge_index"])
    perms = None
    if int(os.environ.get("GCN_BALANCE", "1")):
        from balance import balance_cores
        try:
            perms = balance_cores(edge_index, c.N, c.NCORES, c.NSH,
                                  c.QTILES, c.QSTART)
        except Exception:
            perms = None
    K, bse, NCH, idx_in, S_host, dinv2_in, tog = build_host_data(
        edge_index, c, perms)
    W, B = pack_weights(inputs, c)

    debug = bool(int(os.environ.get("GCN_DEBUG", "0")))
    key = (NCH, tuple(K.ravel().tolist()), debug, c.NQ)
    if key not in _PROGRAM_CACHE:
        _PROGRAM_CACHE[key] = build_program(c, K, bse, NCH, tog, debug)
    nc = _PROGRAM_CACHE[key]

    in_maps = []
    for k in range(c.NCORES):
        xs = np.zeros((c.H, c.NPAD), np.float32)
        xk = x[k * c.NSH:(k + 1) * c.NSH]
        if perms is not None:
            xk = xk[perms[k]]
        xs[:, :c.NSH] = xk.T
        in_maps.append({
            "xT_in": xs,
            "w_in": W,
            "b_in": B,
            "idx_in": np.ascontiguousarray(idx_in[k]),
            "s_in": np.ascontiguousarray(S_host[k]),
            "d2_in": np.ascontiguousarray(dinv2_in[k]),
        })

    trace = bool(int(os.environ.get("GCN_TRACE", "0")))
    res = bass_utils.run_bass_kernel_spmd(
        nc, in_maps, core_ids=list(range(c.NCORES)), trace=trace)
    LAST_RESULTS = res

    out = np.empty((c.N, c.C), np.float32)
    for k in range(c.NCORES):
        o = res.results[k]["out_d"]
        if perms is not None:
            out[k * c.NSH + perms[k]] = o[:, :c.NSH].T
        else:
            out[k * c.NSH:(k + 1) * c.NSH] = o[:, :c.NSH].T
    return out
